# revision 3
# baseline (speedup 1.0000x reference)
"""BTT layer (nn_BTTLayer_36885179138559) as a Trainium2 Bass kernel.

Math: out = x @ W + bias where W[n*64+b, m*64+a] = sum_r btt_r[n,b,m*8+r] *
btt_l[m, n*8+r, a]  (the BTT two-stage contraction collapses to one dense
4096x4096 matmul; W is precomputed on host from the small BTT cores).

Sharding: data-parallel over the flattened batch (4096 rows) across 8
NeuronCores, 512 rows each; W replicated. On-device compute in bf16
(fp32 PSUM accumulation), out returned fp32.
"""

import numpy as np
import ml_dtypes

import concourse.bacc as bacc
import concourse.mybir as mybir
import concourse.tile as tile
import concourse.bass_utils as bass_utils

# problem dims (hardcoded per contract)
M, N, A, B_BLK, RANK = 64, 64, 64, 64, 8
D = 4096              # in = out features
ROWS = 4096           # flattened batch (4, 1024, 4096)
N_CORES = 8
BS = ROWS // N_CORES  # 512 rows per core
KT = 32               # k tiles of 128
OC = 8                # out-column tiles of 512
BT = 4                # batch tiles of 128

BF16 = mybir.dt.bfloat16
F32 = mybir.dt.float32

_compiled = None
_last_in_maps = None


def _build():
    nc = bacc.Bacc("TRN2", target_bir_lowering=False, debug=False, num_devices=N_CORES)
    xt_ap = nc.dram_tensor("xt", [128, KT * BS], BF16, kind="ExternalInput").ap()
    w_ap = nc.dram_tensor("w", [OC, 128, KT * 512], BF16, kind="ExternalInput").ap()
    o_ap = nc.dram_tensor("o", [OC, BT, 128, 512], BF16, kind="ExternalOutput").ap()

    with tile.TileContext(nc) as tc:
        with (
            tc.tile_pool(name="xpool", bufs=1) as xpool,
            tc.tile_pool(name="wpool", bufs=2) as wpool,
            tc.tile_pool(name="opool", bufs=4) as opool,
            tc.tile_pool(name="psum", bufs=6, space="PSUM") as psum,
        ):
            X = xpool.tile([128, KT * BS], BF16)
            nc.sync.dma_start(X[:], xt_ap[:])
            for oc in range(OC):
                Wslab = wpool.tile([128, KT * 512], BF16, tag="w")
                nc.sync.dma_start(Wslab[:], w_ap[oc])
                for bt in range(BT):
                    ps = psum.tile([128, 512], F32, tag="ps")
                    for kt in range(KT):
                        nc.tensor.matmul(
                            ps[:],
                            X[:, kt * BS + bt * 128: kt * BS + (bt + 1) * 128],
                            Wslab[:, kt * 512:(kt + 1) * 512],
                            start=(kt == 0),
                            stop=(kt == KT - 1),
                        )
                    osb = opool.tile([128, 512], BF16, tag="o")
                    nc.scalar.copy(osb[:], ps[:])
                    nc.sync.dma_start(o_ap[oc, bt], osb[:])
    nc.compile()
    return nc


def _get_compiled():
    global _compiled
    if _compiled is None:
        _compiled = _build()
    return _compiled


def kernel(x, btt_r, btt_l, bias):
    x = np.asarray(x)
    btt_r = np.asarray(btt_r)
    btt_l = np.asarray(btt_l)
    bias = np.asarray(bias)
    orig_shape = x.shape

    # ---- host: collapse BTT cores into dense W (fp32) ----
    r4 = btt_r.astype(np.float32).reshape(N, B_BLK, M, RANK)      # [n, b, m, r]
    l4 = btt_l.astype(np.float32).reshape(M, N, RANK, A)          # [m, n, r, a]
    # W[n, b, m, a] = sum_r r4[n,b,m,r] * l4[m,n,r,a]
    W = np.einsum("nbmr,mnra->nbma", r4, l4, optimize=True)
    W = W.reshape(D, D)

    # device W layout: (OC, 128, KT*512); W_dev[oc, kp, kt*512+c] = W[kt*128+kp, oc*512+c]
    W_dev = np.ascontiguousarray(
        W.reshape(KT, 128, OC, 512).transpose(2, 1, 0, 3).reshape(OC, 128, KT * 512)
    ).astype(ml_dtypes.bfloat16)

    # per-core x shards, transposed: X_dev[kp, kt*BS + col] = xs[col, kt*128+kp]
    xr = x.astype(np.float32).reshape(ROWS, D)
    in_maps = []
    for c in range(N_CORES):
        xs = xr[c * BS:(c + 1) * BS]                               # (BS, D)
        xt = np.ascontiguousarray(
            xs.T.reshape(KT, 128, BS).transpose(1, 0, 2).reshape(128, KT * BS)
        ).astype(ml_dtypes.bfloat16)
        in_maps.append({"xt": xt, "w": W_dev})

    global _last_in_maps
    _last_in_maps = in_maps
    nc = _get_compiled()
    res = bass_utils.run_bass_kernel_spmd(nc, in_maps, core_ids=list(range(N_CORES)))

    # ---- gather: o (OC, BT, 128, 512) -> rows (BS, D) per core ----
    out = np.empty((ROWS, D), dtype=np.float32)
    for c in range(N_CORES):
        o = res.results[c]["o"].astype(np.float32)                  # (OC, BT, 128, 512)
        # out[bt*128+p, oc*512+c2] = o[oc, bt, p, c2]
        out[c * BS:(c + 1) * BS] = o.transpose(1, 2, 0, 3).reshape(BS, D)
    out += bias.astype(np.float32)[None, :]
    return out.reshape(*orig_shape[:-1], D)


# revision 4
# speedup vs baseline: 1.0199x; 1.0199x over previous
"""BTT layer (nn_BTTLayer_36885179138559) as a Trainium2 Bass kernel.

Math: out = x @ W + bias where W[n*64+b, m*64+a] = sum_r btt_r[n,b,m*8+r] *
btt_l[m, n*8+r, a]  (the BTT two-stage contraction collapses to one dense
4096x4096 matmul; W is precomputed on host from the small BTT cores).

Sharding: data-parallel over the flattened batch (4096 rows) across 8
NeuronCores, 512 rows each; W replicated. On-device compute in bf16
(fp32 PSUM accumulation), out returned fp32.
"""

import numpy as np
import ml_dtypes

import concourse.bacc as bacc
import concourse.mybir as mybir
import concourse.tile as tile
import concourse.bass_utils as bass_utils

# problem dims (hardcoded per contract)
M, N, A, B_BLK, RANK = 64, 64, 64, 64, 8
D = 4096              # in = out features
ROWS = 4096           # flattened batch (4, 1024, 4096)
N_CORES = 8
BS = ROWS // N_CORES  # 512 rows per core
KT = 32               # k tiles of 128
OC = 8                # out-column tiles of 512
BT = 4                # batch tiles of 128

BF16 = mybir.dt.bfloat16
F32 = mybir.dt.float32

_compiled = None
_last_in_maps = None


def _build():
    nc = bacc.Bacc("TRN2", target_bir_lowering=False, debug=False, num_devices=N_CORES)
    xt_ap = nc.dram_tensor("xt", [128, KT * BS], BF16, kind="ExternalInput").ap()
    w_ap = nc.dram_tensor("w", [OC, 128, KT * 512], BF16, kind="ExternalInput").ap()
    o_ap = nc.dram_tensor("o", [OC, BT, 128, 512], BF16, kind="ExternalOutput").ap()

    XG = 8          # x split into 8 column-group tiles (4 kt each)
    WS = 4          # each W slab split into 4 sub-tiles (8 kt each)
    with tile.TileContext(nc) as tc:
        with (
            tc.tile_pool(name="xpool", bufs=1) as xpool,
            tc.tile_pool(name="wpool", bufs=2 * WS) as wpool,
            tc.tile_pool(name="opool", bufs=4) as opool,
            tc.tile_pool(name="psum", bufs=6, space="PSUM") as psum,
        ):
            xg_tiles = []
            for g in range(XG):
                Xg = xpool.tile([128, (KT // XG) * BS], BF16, tag=f"x{g}")
                nc.sync.dma_start(Xg[:], xt_ap[:, g * (KT // XG) * BS:(g + 1) * (KT // XG) * BS])
                xg_tiles.append(Xg)
            for oc in range(OC):
                wsubs = []
                for s in range(WS):
                    Wsub = wpool.tile([128, (KT // WS) * 512], BF16, tag="w")
                    nc.sync.dma_start(
                        Wsub[:], w_ap[oc][:, s * (KT // WS) * 512:(s + 1) * (KT // WS) * 512]
                    )
                    wsubs.append(Wsub)
                for bt in range(BT):
                    ps = psum.tile([128, 512], F32, tag="ps")
                    for kt in range(KT):
                        Xg = xg_tiles[kt // (KT // XG)]
                        xoff = (kt % (KT // XG)) * BS + bt * 128
                        Wsub = wsubs[kt // (KT // WS)]
                        woff = (kt % (KT // WS)) * 512
                        nc.tensor.matmul(
                            ps[:],
                            Xg[:, xoff:xoff + 128],
                            Wsub[:, woff:woff + 512],
                            start=(kt == 0),
                            stop=(kt == KT - 1),
                        )
                    osb = opool.tile([128, 512], BF16, tag="o")
                    nc.scalar.copy(osb[:], ps[:])
                    nc.sync.dma_start(o_ap[oc, bt], osb[:])
    nc.compile()
    return nc


def _get_compiled():
    global _compiled
    if _compiled is None:
        _compiled = _build()
    return _compiled


def kernel(x, btt_r, btt_l, bias):
    x = np.asarray(x)
    btt_r = np.asarray(btt_r)
    btt_l = np.asarray(btt_l)
    bias = np.asarray(bias)
    orig_shape = x.shape

    # ---- host: collapse BTT cores into dense W (fp32) ----
    r4 = btt_r.astype(np.float32).reshape(N, B_BLK, M, RANK)      # [n, b, m, r]
    l4 = btt_l.astype(np.float32).reshape(M, N, RANK, A)          # [m, n, r, a]
    # W[n, b, m, a] = sum_r r4[n,b,m,r] * l4[m,n,r,a]
    W = np.einsum("nbmr,mnra->nbma", r4, l4, optimize=True)
    W = W.reshape(D, D)

    # device W layout: (OC, 128, KT*512); W_dev[oc, kp, kt*512+c] = W[kt*128+kp, oc*512+c]
    W_dev = np.ascontiguousarray(
        W.reshape(KT, 128, OC, 512).transpose(2, 1, 0, 3).reshape(OC, 128, KT * 512)
    ).astype(ml_dtypes.bfloat16)

    # per-core x shards, transposed: X_dev[kp, kt*BS + col] = xs[col, kt*128+kp]
    xr = x.astype(np.float32).reshape(ROWS, D)
    in_maps = []
    for c in range(N_CORES):
        xs = xr[c * BS:(c + 1) * BS]                               # (BS, D)
        xt = np.ascontiguousarray(
            xs.T.reshape(KT, 128, BS).transpose(1, 0, 2).reshape(128, KT * BS)
        ).astype(ml_dtypes.bfloat16)
        in_maps.append({"xt": xt, "w": W_dev})

    global _last_in_maps
    _last_in_maps = in_maps
    nc = _get_compiled()
    res = bass_utils.run_bass_kernel_spmd(nc, in_maps, core_ids=list(range(N_CORES)))

    # ---- gather: o (OC, BT, 128, 512) -> rows (BS, D) per core ----
    out = np.empty((ROWS, D), dtype=np.float32)
    for c in range(N_CORES):
        o = res.results[c]["o"].astype(np.float32)                  # (OC, BT, 128, 512)
        # out[bt*128+p, oc*512+c2] = o[oc, bt, p, c2]
        out[c * BS:(c + 1) * BS] = o.transpose(1, 2, 0, 3).reshape(BS, D)
    out += bias.astype(np.float32)[None, :]
    return out.reshape(*orig_shape[:-1], D)
